# revision 32
# baseline (speedup 1.0000x reference)
"""Trainium2 Bass kernel for the NP/NY/NU RNN scan (nn_BlackBoxModel_24489903521937).

Model (per step t, batch row b):
    x_t   = [y_t, y_{t-4..t-1}, u_{t-4..t-1}, u_t]          (60)
    h1    = tanh(x_t @ W1 + b1)                              (128)
    h2    = tanh(h1 @ W2 + b2)                               (128)
    y_{t+1} = h2 @ W3 + b3                                   (8)
    output ys[:, t] = y_t

Strategy (pure data parallel, batch 4096 -> 8 cores x 512 columns):
  * feature-major layout: features on SBUF partitions, batch on the free dim.
    y-history lives in 4 ring slots of a [128, 512] staging tile (strips at
    partitions 0/32/64/96); x @ W1 becomes one K=128 matmul against
    phase-permuted W1 blocks (mm1; the K=20 u-window map is folded into the
    same matrices, with u values DMA'd into stag's free partitions each
    step) and one composed (W3 @ A0) matmul from h2 (mmC), so the recurrent
    chain is just tanh1 -> mm2 -> tanh2 -> mmC.
  * 2-chunk software pipeline: the batch columns are split in halves a/b and
    every chain op is issued per half (tanh1a, tanh1b, mm2a, mm2b, tanh2a,
    tanh2b, mmCa, mmCb).  Each half's preactivations live in their own PSUM
    banks (8 banks: ph1a x2, ph1b x2, ph2a, ph2b, pypa, pypb) so a reader of
    one half never waits on the other half's accumulation group (Tile makes
    readers wait for a group's stop instruction, and a matmul start clears
    the whole bank's has_written bits -- one start/stop pair per bank).
    The scalar engine then runs 4 back-to-back 474 ns tanh ACTIVATEs per
    step (94% busy); all matmuls hide underneath.
  * biases: ACTIVATE with an AP bias costs ~90 ns extra per instruction, so
    when b1/b2 are all-zero (this problem instance) a zero_bias build uses
    immediate-0 bias; the general build keeps the AP path.
  * y_{t-4} is read from the slot y_t is about to overwrite (emission
    order makes Tile sequence the write after the read).
  * a ~5 us burst of back-to-back scratch matmuls at kernel start trips the
    PE HAM clock gate from 1.2 GHz to 2.4 GHz before the recurrence starts;
    outputs retire by one 8-row slot DMA per step; the host transposes
    [T,8,B] -> [B,T,8] at the end.
  * matmul operands are fp16 (1 cycle/row, fp32 PSUM accumulate); the
    fading memory of the state keeps fp16 error flat (~6e-4).
"""

import numpy as np

NP_, NY, NU = 4, 8, 4
B, T, H = 4096, 256, 128
NCORES = 8
BC = B // NCORES  # 512 batch rows per core
CHUNKS = 2        # column chunks for the critical tanh/matmul cycle
CW = BC // CHUNKS
NSLOT = 4         # y ring slots (one per 32-partition strip)

_COMPILED = {}


def _build_program(zero_bias):
    import concourse.mybir as mybir
    import concourse.tile as tile
    from concourse import bacc

    f32 = mybir.dt.float32
    fh = mybir.dt.float16
    Tanh = mybir.ActivationFunctionType.Tanh

    nc = bacc.Bacc("TRN2", target_bir_lowering=False, debug=False)

    d_stag0 = nc.dram_tensor("stag0", [128, BC], fh, kind="ExternalInput")
    d_u16 = nc.dram_tensor("u16", [T, 16, BC], fh, kind="ExternalInput")
    d_u4 = nc.dram_tensor("u4", [T, 4, BC], fh, kind="ExternalInput")
    # packed blob: 8 C matrices (phases 0..3 + boot 0..3), then W2, WC, W3
    d_wblob = nc.dram_tensor("wblob", [128, 8 * 128 + 264], fh, kind="ExternalInput")
    if not zero_bias:
        d_b1 = nc.dram_tensor("b1v", [128, 1], f32, kind="ExternalInput")
        d_b1b = nc.dram_tensor("b1b", [128, 1], f32, kind="ExternalInput")
        d_b2 = nc.dram_tensor("b2v", [128, 1], f32, kind="ExternalInput")
    d_b3 = nc.dram_tensor("b3v", [8, 1], f32, kind="ExternalInput")
    d_out2 = nc.dram_tensor("out2", [T // 4, 4, 8, BC], fh, kind="ExternalOutput")
    d_warm = nc.dram_tensor("warm", [8, 16], fh, kind="ExternalOutput")

    with tile.TileContext(nc) as tc:
        with (
            tc.tile_pool(name="const", bufs=1) as cpool,
            tc.tile_pool(name="stagp", bufs=1) as spool,
            tc.tile_pool(name="hpool", bufs=2) as hpool,
            tc.tile_pool(name="ph1a", bufs=2, space="PSUM") as ph1ap,
            tc.tile_pool(name="ph1b", bufs=2, space="PSUM") as ph1bp,
            tc.tile_pool(name="ph2a", bufs=1, space="PSUM") as ph2ap,
            tc.tile_pool(name="ph2b", bufs=1, space="PSUM") as ph2bp,
            tc.tile_pool(name="pypa", bufs=1, space="PSUM") as pypap,
            tc.tile_pool(name="pypb", bufs=1, space="PSUM") as pypbp,
        ):
            t_wb = cpool.tile_from(d_wblob[:])
            t_cm = t_wb[:, 0:1024]
            t_w2 = t_wb[:, 1024:1152]
            t_wc = t_wb[:, 1152:1280]
            t_w3 = t_wb[:, 1280:1288]
            if not zero_bias:
                t_b1 = cpool.tile_from(d_b1[:])
                t_b1b = cpool.tile_from(d_b1b[:])
                t_b2 = cpool.tile_from(d_b2[:])
            t_b3 = cpool.tile_from(d_b3[:])
            # scratch operands for the PE warm-up so it can run concurrently
            # with the input DMAs (values are discarded); gpsimd memset keeps
            # the tile allocator happy without touching the DMA queues
            t_scr = cpool.tile([128, BC], fh, name="scr")
            nc.gpsimd.memset(t_scr[:, :], 0.0)

            stag = spool.tile([128, BC], fh, name="stag")
            nc.sync.dma_start(stag[:], d_stag0[:])

            # --- PE clock warm-up: ~6us of back-to-back matmuls trips the
            # HAM clock gate from 1.2 GHz (cold K=4/8) to 2.4 GHz before the
            # recurrence starts.  Results land in a scratch PSUM tile whose
            # corner is exported so the chain stays live.
            warm_p = ph2ap.tile([128, BC], f32, name="warmp", tag="h2pa")
            for _ in range(10):
                nc.tensor.matmul(
                    warm_p[:, :], t_scr[:, 0:128], t_scr[:, :],
                    start=True, stop=True, skip_group_check=True,
                )
            warm_s = cpool.tile([8, 16], fh, name="warms")
            nc.scalar.copy(warm_s[:, :], warm_p[0:8, 0:16])
            nc.sync.dma_start(d_warm[:], warm_s[:, :])

            def cmat(i):
                return t_wb[:, 128 * i:128 * i + 128]

            CA = slice(0, CW)
            CB = slice(CW, BC)

            def emit_group_xu(tt, phA, phB):
                """Open both half groups for step tt.  The u-window term is
                folded into the phase matrices (u values live in stag's free
                partitions 16:32 and 112:116, refreshed by per-step DMAs), so
                one K=128 matmul per half covers y-history + u-window.
                start=True is that bank's has_written clear; the half's mmC
                closes the group (at boot the group is just this matmul)."""
                cidx = 4 + tt if tt < 4 else tt % NSLOT
                for ph, cs in ((phA, CA), (phB, CB)):
                    nc.tensor.matmul(
                        ph[:, :],
                        cmat(cidx),
                        stag[:, cs],
                        start=True, stop=(tt == 0), skip_group_check=True,
                    )

            def flush(ty):
                """Export y_{ty} (slot ty%4) feature-major to DRAM; the host
                transposes to batch-major at the end."""
                s = ty % 4
                nc.gpsimd.dma_start(d_out2[ty // 4, s], stag[32 * s:32 * s + 8, :])

            phA_cur = ph1ap.tile([128, CW], f32, name="h1pa", tag="h1pa")
            phB_cur = ph1bp.tile([128, CW], f32, name="h1pb", tag="h1pb")
            emit_group_xu(0, phA_cur, phB_cur)

            for t in range(T):
                if zero_bias:
                    bias1 = bias2 = 0.0
                else:
                    b1t = t_b1b if t == 0 else t_b1
                    bias1 = b1t[:, 0:1]
                    bias2 = t_b2[:, 0:1]

                # --- tanh1 chunks 1a, 1b ---
                h1_t = hpool.tile([128, BC], fh, name="h1", tag="h1")
                nc.scalar.activation(
                    h1_t[:, CA], phA_cur[:, :], Tanh, bias=bias1
                )
                nc.scalar.activation(
                    h1_t[:, CB], phB_cur[:, :], Tanh, bias=bias1
                )

                # --- mm2 per half (separate PSUM banks so tanh2a only
                #     depends on mm2a) ---
                ph2a_t = ph2ap.tile([128, CW], f32, name="h2pa", tag="h2pa")
                ph2b_t = ph2bp.tile([128, CW], f32, name="h2pb", tag="h2pb")
                nc.tensor.matmul(ph2a_t[:, :], t_w2[:, :], h1_t[:, CA])
                nc.tensor.matmul(ph2b_t[:, :], t_w2[:, :], h1_t[:, CB])

                # --- refresh the u-window rows of stag for step t+1
                #     (emitted after mm1(t) [last iteration] so Tile orders
                #     the DMA behind mm1(t)'s read of window t) ---
                if t + 1 < T:
                    nc.sync.dma_start(stag[16:32, :], d_u16[t + 1])
                    nc.gpsimd.dma_start(stag[112:116, :], d_u4[t + 1])

                # --- open next step's half groups (off the critical chain;
                #     emitted before this step's staging writes so the stale
                #     y_{t-3} slot read stays dependency-free) ---
                phA_next = phB_next = None
                if t + 1 < T:
                    phA_next = ph1ap.tile([128, CW], f32, name="h1pa", tag="h1pa")
                    phB_next = ph1bp.tile([128, CW], f32, name="h1pb", tag="h1pb")
                    emit_group_xu(t + 1, phA_next, phB_next)

                # --- tanh2 chunks 2a, 2b ---
                h2_t = hpool.tile([128, BC], fh, name="h2", tag="h2")
                nc.scalar.activation(
                    h2_t[:, CA], ph2a_t[:, :], Tanh, bias=bias2
                )
                nc.scalar.activation(
                    h2_t[:, CB], ph2b_t[:, :], Tanh, bias=bias2
                )

                # --- output flush (1 slot/step; ~3 steps of slack) ---
                if t >= 1:
                    flush(t - 1)

                # --- close the half groups + y_{t+1} = W3^T h2 + b3:
                #     PE order mmCa, mm3a, mm3b, mmCb ---
                if t + 1 < T:
                    pypa_t = pypap.tile([8, CW], f32, name="ypa", tag="ypa")
                    pypb_t = pypbp.tile([8, CW], f32, name="ypb", tag="ypb")
                    nc.tensor.matmul(
                        phA_next[:, :], t_wc[:, :], h2_t[:, CA],
                        start=False, stop=True, skip_group_check=True,
                    )
                    nc.tensor.matmul(pypa_t[:, :], t_w3[:, :], h2_t[:, CA])
                    nc.tensor.matmul(
                        phB_next[:, :], t_wc[:, :], h2_t[:, CB],
                        start=False, stop=True, skip_group_check=True,
                    )
                    nc.tensor.matmul(pypb_t[:, :], t_w3[:, :], h2_t[:, CB])
                    p0 = 32 * ((t + 1) % NSLOT)
                    nc.vector.tensor_scalar_add(
                        stag[p0:p0 + 8, CA], pypa_t[:, :], t_b3[:, 0:1]
                    )
                    nc.vector.tensor_scalar_add(
                        stag[p0:p0 + 8, CB], pypb_t[:, :], t_b3[:, 0:1]
                    )

                phA_cur = phA_next
                phB_cur = phB_next

            flush(T - 1)

    nc.compile()
    return nc


def _host_prep(useq, yz0, W1, b1, W2, b2, W3, b3):
    """Build the per-core input maps (all host-side numpy)."""
    useq = np.ascontiguousarray(useq, dtype=np.float32)
    yz0 = np.ascontiguousarray(yz0, dtype=np.float32)
    W1 = np.asarray(W1, dtype=np.float32)
    W2 = np.ascontiguousarray(W2, dtype=np.float32)
    W3 = np.ascontiguousarray(W3, dtype=np.float32)
    b1 = np.asarray(b1, dtype=np.float32)
    b2 = np.asarray(b2, dtype=np.float32)
    b3 = np.asarray(b3, dtype=np.float32)

    A = {0: W1[0:8], 4: W1[8:16], 3: W1[16:24], 2: W1[24:32], 1: W1[32:40]}
    Bstack = W1[40:60]  # u_{t-4..t} stacked chronologically

    # staging rows: slot s -> [32s, 32s+8) holds y ring;
    #               boot block s -> [32s+8, 32s+16) holds y_{-(s+1)}
    cmats = np.zeros((8, 128, 128), dtype=np.float32)
    for p in range(NSLOT):  # steady phases, t >= 4: every slot one A_k
        for s in range(NSLOT):
            k = ((p - s - 1) % 4) + 1
            cmats[p, 32 * s:32 * s + 8] = A[k]
    for tt in range(4):  # boot steps t=0..3
        cb = cmats[4 + tt]
        for k in range(1, 5):
            if tt - k >= 0:
                s = (tt - k) % 4
                cb[32 * s:32 * s + 8] += A[k]
            else:
                s = k - tt - 1
                cb[32 * s + 8:32 * s + 16] += A[k]
        if tt == 0:
            cb[0:8] += A[0]  # slot 0 carries y_0 directly at t=0
    # fold the u-window map into every phase/boot matrix: u values live in
    # stag partitions 16:32 (Bstack rows 0:16) and 112:116 (rows 16:20)
    for i in range(8):
        cmats[i][16:32] = Bstack[0:16]
        cmats[i][112:116] = Bstack[16:20]
    cmats2d = np.ascontiguousarray(
        cmats.transpose(1, 0, 2).reshape(128, 8 * 128)
    )

    WC = np.ascontiguousarray(W3 @ A[0])          # [128, 128]
    b1_eff = (b1 + A[0].T @ b3).reshape(128, 1)   # mmC path lacks A0^T b3
    b1_boot = b1.reshape(128, 1)
    b2v = b2.reshape(128, 1)
    b3v = b3.reshape(8, 1)
    ident = np.eye(128, dtype=np.float16)

    in_maps = []
    for c in range(NCORES):
        bs = slice(c * BC, (c + 1) * BC)
        u_c = useq[bs]      # [BC, T, 4]
        yz_c = yz0[bs]      # [BC, 56]

        stag0 = np.zeros((128, BC), dtype=np.float32)
        stag0[0:8] = yz_c[:, 0:8].T               # slot 0 = y_0
        for s in range(4):                         # boot blocks y_{-(s+1)}
            blk = yz_c[:, 8 + 8 * (3 - s):16 + 8 * (3 - s)]  # ypseq newest last
            stag0[32 * s + 8:32 * s + 16] = blk.T

        # sliding u-windows for the K=20 u matmul
        uhist = yz_c[:, 40:56].reshape(BC, 4, 4)          # u_{-4..-1}
        uext = np.concatenate([uhist, u_c], axis=1)       # [BC, T+4, 4]
        sw = np.lib.stride_tricks.sliding_window_view(uext, 5, axis=1)
        # sw: [BC, T, 4, 5] -> uwin [T, 20, BC] (chronological rows)
        uwin = np.ascontiguousarray(sw.transpose(1, 3, 2, 0).reshape(T, 20, BC))

        stag0[16:32] = uwin[0][0:16]              # u-window for step 0
        stag0[112:116] = uwin[0][16:20]
        wblob = np.concatenate([cmats2d, W2, WC, W3], axis=1)
        in_maps.append({
            "stag0": stag0.astype(np.float16),
            "u16": np.ascontiguousarray(uwin[:, 0:16]).astype(np.float16),
            "u4": np.ascontiguousarray(uwin[:, 16:20]).astype(np.float16),
            "wblob": np.ascontiguousarray(wblob).astype(np.float16),
            "b1v": np.ascontiguousarray(b1_eff),
            "b1b": np.ascontiguousarray(b1_boot),
            "b2v": np.ascontiguousarray(b2v),
            "b3v": np.ascontiguousarray(b3v),
        })
    return in_maps


def get_program(zero_bias=False):
    key = ("nc", bool(zero_bias))
    if key not in _COMPILED:
        _COMPILED[key] = _build_program(zero_bias)
    return _COMPILED[key]


def _enable_ldw_opt():
    """Allow walrus to double-buffer LDWEIGHTS (background weight loads).

    The environment default is --enable-ldw-opt=false, which serializes
    every LDWEIGHTS behind the previous matmul's drain; with ~6 weight
    switches per RNN step that costs ~2x on the tensor engine.
    """
    try:
        from concourse.compiler_utils import get_compiler_flags, set_compiler_flags

        flags = get_compiler_flags()
        new = [f.replace("--enable-ldw-opt=false", "--enable-ldw-opt=true") for f in flags]
        if new != flags:
            set_compiler_flags(new)
    except Exception:
        pass


def run_cores(in_maps, zero_bias=False, **kwargs):
    from concourse.bass_utils import run_bass_kernel_spmd

    _enable_ldw_opt()
    nc = get_program(zero_bias)
    if zero_bias:
        drop = {"b1v", "b1b", "b2v"}
        in_maps = [{k: v for k, v in m.items() if k not in drop} for m in in_maps]
    return run_bass_kernel_spmd(nc, in_maps, core_ids=list(range(NCORES)), **kwargs)


def assemble(res):
    outs = []
    for r in res.results:
        buf = np.asarray(r["out2"], dtype=np.float32)   # [T/4, 4, 8, BC]
        ys = buf.transpose(3, 0, 1, 2).reshape(BC, T, NY)
        outs.append(ys)
    return np.concatenate(outs, axis=0)


def kernel(useq, yz0, W1, b1, W2, b2, W3, b3):
    in_maps = _host_prep(useq, yz0, W1, b1, W2, b2, W3, b3)
    zb = bool(
        np.all(in_maps[0]["b1v"] == 0.0)
        and np.all(in_maps[0]["b1b"] == 0.0)
        and np.all(in_maps[0]["b2v"] == 0.0)
    )
    res = run_cores(in_maps, zero_bias=zb)
    return assemble(res)

